# revision 1
# baseline (speedup 1.0000x reference)
"""Trainium2 Bass kernel for BlockdiagButterflyLinear.

Computes y = butterfly(x; w1, w2) + bias where
  tmp[b,k,j,y] = sum_i x[b, k*1024+i] * w1[k, j*48+y, i]
  out[b, 4l+j] = sum_{k,y} tmp[b,k,j,y] * w2[j, l, k*48+y] + bias[4l+j]

Sharding: data-parallel over the 8192 token rows across 8 NeuronCores
(1024 tokens/core); the small butterfly factors are replicated.

Per-core pipeline (four 256-token chunks):
  1. DMA x in [128 token, 1024 feat] tiles (one per (chunk, k, m)).
  2. PE transpose (identity matmul) -> x^T tiles [i, token] in SBUF.
  3. Stage 1 matmuls (float32r, full-rate): [48f, 256tok] PSUM tile per
     (k, j), contracting i over 8x128.
  4. Copies PSUM->SBUF build stage-2 lhsT tiles t2[j][c] of 113
     partitions: rows 0:48 = k=2c, rows 64:112 = k=2c+1, zero gap rows
     48:64, constant-one row 112 (bias folded into stage-2 weights).
     All partition starts are in {0, 32, 64, 96} per the engine rule.
  5. Stage 2 matmuls: out[tok, l] accumulating the two 113-row ky
     chunks; result copied with stride-4 interleave into the output tile.
  6. DMA out [128, 2048] halves as soon as their four j-copies land.
"""

import sys

sys.path.insert(0, "/opt/trn_rl_repo")

from contextlib import ExitStack

import numpy as np

import concourse.bacc as bacc
import concourse.bass as bass
import concourse.mybir as mybir
import concourse.tile as tile
from concourse.bass_utils import run_bass_kernel_spmd
from concourse.masks import make_identity

F32 = mybir.dt.float32
F32R = mybir.dt.float32r

N_CORES = 8
TOK_PER_CORE = 1024  # 8192 tokens / 8 cores
N_FEAT = 4096
K, J, B1 = 4, 4, 48
F = J * B1  # 192 rows out of stage 1 per k-block
CH = 256  # token chunk
MB = CH // 128  # 128-token subchunks per chunk
N_CH = TOK_PER_CORE // CH
L = 1024  # l dim of stage 2 per j
R2 = 113  # stage-2 contraction rows: 48 + 16 gap + 48 + 1 ones row

_PROGRAM = None


def _build_program() -> bass.Bass:
    nc = bacc.Bacc(None, target_bir_lowering=False)
    xs = nc.declare_dram_parameter("xs", [TOK_PER_CORE, N_FEAT], F32, isOutput=False)
    w1t = nc.declare_dram_parameter("w1t", [K, 1024, F], F32, isOutput=False)
    w2tb = nc.declare_dram_parameter("w2tb", [J, 2, R2, L], F32, isOutput=False)
    out = nc.declare_dram_parameter("out", [TOK_PER_CORE, N_FEAT], F32, isOutput=True)

    with ExitStack() as ctx:
        tc = ctx.enter_context(tile.TileContext(nc))
        consts = ctx.enter_context(tc.tile_pool(name="consts", bufs=1))
        wpool = ctx.enter_context(tc.tile_pool(name="wpool", bufs=1))
        xpool = ctx.enter_context(tc.tile_pool(name="xpool", bufs=12))
        xtpool = ctx.enter_context(tc.tile_pool(name="xtpool", bufs=2))
        outpool = ctx.enter_context(tc.tile_pool(name="outpool", bufs=4))
        ptpool = ctx.enter_context(tc.tile_pool(name="ptpool", bufs=2, space="PSUM"))
        p1pool = ctx.enter_context(tc.tile_pool(name="p1pool", bufs=2, space="PSUM"))
        p2pool = ctx.enter_context(tc.tile_pool(name="p2pool", bufs=2, space="PSUM"))

        identf = consts.tile([128, 128], F32)
        make_identity(nc, identf)
        # f32r-rounded identity so the x transposes can run in f32r mode
        # (1.5 PE cycles/row instead of 2.0 for fp32); values are exact.
        ident = consts.tile([128, 128], F32R)
        nc.scalar.copy(ident[:], identf[:])

        # constant rows for the t2 tiles (copied in with f32r rounding)
        zrows = consts.tile([32, CH], F32)
        nc.any.memset(zrows[:], 0.0)
        orows = consts.tile([17, CH], F32)
        nc.any.memset(orows[:], 1.0)

        # Resident butterfly factors, split per k / per j so their DMAs can
        # interleave with the first x-tile loads (keeps the x stream dense
        # at startup): w1sk[k][p, ic, f] = w1t[k, ic*128+p, f] and
        # w2sj[j][r, c, l] = w2tb[j, c, r, l].
        w1sk = [wpool.tile([128, 8, F], F32R, name=f"w1s_{k}") for k in range(K)]
        w2sj = [wpool.tile([R2, 2, L], F32R, name=f"w2s_{j}") for j in range(J)]

        def load_w1(k):
            nc.sync.dma_start(
                w1sk[k][:],
                w1t[k].bitcast(F32R).rearrange("(ic p) f -> p ic f", p=128),
            )

        def load_w2(j):
            nc.sync.dma_start(
                w2sj[j][:], w2tb[j].bitcast(F32R).rearrange("c r l -> r c l")
            )

        # Stage-2 lhsT tiles, statically double-buffered by chunk parity:
        # rows 0:48 = (k=2c), 64:112 = (k=2c+1), 112 = ones. The constant
        # gap/ones rows are initialized once per physical tile.
        t2s = [
            [
                [
                    consts.tile([R2, CH], F32R, name=f"t2_{j}_{c}_{par}")
                    for par in range(2)
                ]
                for c in range(2)
            ]
            for j in range(J)
        ]
        for j in range(J):
            for c in range(2):
                for par in range(2):
                    nc.scalar.copy(t2s[j][c][par][32:64, :], zrows[:])
                    nc.scalar.copy(t2s[j][c][par][96:R2, :], orows[:])

        xtiles = {}

        def load_x(ch):
            # x DMAs for one chunk; chunks 0/1 interleave the w1/w2 loads
            # into the x stream so the PE can start transposing immediately
            for k in range(K):
                tiles = []
                for m in range(MB):
                    xm = xpool.tile(
                        [128, 1024], F32R, tag="xk", name=f"x_{ch}_{k}_{m}"
                    )
                    row0 = ch * CH + m * 128
                    if ch == 0 and k == 0:
                        # two half loads so the first transposes start sooner
                        for h in range(2):
                            nc.sync.dma_start(
                                xm[:, h * 512 : (h + 1) * 512],
                                xs[
                                    row0 : row0 + 128,
                                    k * 1024 + h * 512 : k * 1024 + (h + 1) * 512,
                                ].bitcast(F32R),
                            )
                    else:
                        nc.sync.dma_start(
                            xm[:],
                            xs[
                                row0 : row0 + 128, k * 1024 : (k + 1) * 1024
                            ].bitcast(F32R),
                        )
                    tiles.append(xm)
                    if ch == 0 and m == 1:
                        load_w1(k)
                    if ch == 1 and m == 1:
                        load_w2(k)
                xtiles[(ch, k)] = tiles

        load_x(0)
        deferred = []
        for ch in range(N_CH):
            t2 = [[t2s[j][c][ch % 2] for c in range(2)] for j in range(J)]

            for k in range(K):
                xk = xtiles[(ch, k)]
                # xt[i, ic, tok] with tok = m*128 + p
                xt = xtpool.tile([128, 8, CH], F32R, tag="xt")
                for m in range(MB):
                    xtp = ptpool.tile([128, 8, 128], F32R, tag="xtp")
                    for ic in range(8):
                        nc.tensor.transpose(
                            xtp[:, ic, :],
                            xk[m][:, ic * 128 : (ic + 1) * 128],
                            ident[:],
                        )
                    if ch == 0:
                        # two half copies (both DVE) so the copy overlaps the
                        # second half of the transposes during warmup
                        nc.vector.tensor_copy(
                            xt[:, 0:4, m * 128 : (m + 1) * 128], xtp[:, 0:4, :]
                        )
                        nc.vector.tensor_copy(
                            xt[:, 4:8, m * 128 : (m + 1) * 128], xtp[:, 4:8, :]
                        )
                    else:
                        nc.vector.tensor_copy(
                            xt[:, :, m * 128 : (m + 1) * 128], xtp[:]
                        )
                for j in range(J):
                    p1 = p1pool.tile([48, CH], F32, tag="p1")
                    for ic in range(8):
                        nc.tensor.matmul(
                            p1[:],
                            w1sk[k][:, ic, j * 48 : (j + 1) * 48],
                            xt[:, ic, :],
                            start=(ic == 0),
                            stop=(ic == 7),
                        )
                    r0 = (k % 2) * 64
                    nc.scalar.copy(t2[j][k // 2][r0 : r0 + 48, :], p1[:])

            if ch + 1 < N_CH:
                load_x(ch + 1)
            for m in range(MB):
                outm = outpool.tile([128, L, 4], F32, tag="outm")
                row0 = ch * CH + m * 128
                for lc in range(2):
                    for j in range(J):
                        p2 = p2pool.tile([128, 512], F32, tag="p2")
                        for c in range(2):
                            nc.tensor.matmul(
                                p2[:],
                                t2[j][c][:, m * 128 : (m + 1) * 128],
                                w2sj[j][:, c, lc * 512 : (lc + 1) * 512],
                                start=(c == 0),
                                stop=(c == 1),
                            )
                        oslice = outm[:, lc * 512 : (lc + 1) * 512, j]
                        if j % 2 == 0:
                            nc.vector.tensor_copy(oslice, p2[:])
                        else:
                            nc.scalar.copy(oslice, p2[:])
                    if ch == 0 and m == 0:
                        # deferred to program end: fills the SP idle window
                        # while the last chunk's stage 2 finishes
                        deferred.append((row0, lc, outm))
                    elif ch == N_CH - 1:
                        # last chunk: quarter-granularity stores shorten the
                        # trailing DMA after the final copies
                        for q in range(2):
                            nc.sync.dma_start(
                                out[
                                    row0 : row0 + 128,
                                    lc * 2048 + q * 1024 : lc * 2048 + (q + 1) * 1024,
                                ],
                                outm[:, lc * 512 + q * 256 : lc * 512 + (q + 1) * 256, :],
                            )
                    else:
                        nc.sync.dma_start(
                            out[row0 : row0 + 128, lc * 2048 : (lc + 1) * 2048],
                            outm[:, lc * 512 : (lc + 1) * 512, :],
                        )

        for row0, lc, outm in deferred:
            nc.sync.dma_start(
                out[row0 : row0 + 128, lc * 2048 : (lc + 1) * 2048],
                outm[:, lc * 512 : (lc + 1) * 512, :],
            )

    nc.compile()
    nc.finalize()
    return nc


def _get_program() -> bass.Bass:
    global _PROGRAM
    if _PROGRAM is None:
        _PROGRAM = _build_program()
    return _PROGRAM


def _prep_weights(w1, w2, b):
    w1t = np.ascontiguousarray(w1.transpose(0, 2, 1))  # (4, 1024, 192)
    w2tb = np.zeros((J, 2, R2, L), np.float32)
    for j in range(J):
        for c in range(2):
            w2tb[j, c, 0:48, :] = w2[j][:, (2 * c) * 48 : (2 * c) * 48 + 48].T
            w2tb[j, c, 64:112, :] = w2[j][:, (2 * c + 1) * 48 : (2 * c + 1) * 48 + 48].T
        w2tb[j, 1, 112, :] = b[j :: J]  # bias[4l+j]
    return w1t, w2tb


def kernel(x, w1_bfly, w2_bfly, bias):
    x = np.asarray(x, dtype=np.float32)
    w1 = np.asarray(w1_bfly, dtype=np.float32)
    w2 = np.asarray(w2_bfly, dtype=np.float32)
    b = np.asarray(bias, dtype=np.float32)

    x_shape = x.shape
    xf = np.ascontiguousarray(x).reshape(-1, N_FEAT)
    w1t, w2tb = _prep_weights(w1, w2, b)

    nc = _get_program()
    in_maps = [
        {
            "xs": np.ascontiguousarray(xf[c * TOK_PER_CORE : (c + 1) * TOK_PER_CORE]),
            "w1t": w1t,
            "w2tb": w2tb,
        }
        for c in range(N_CORES)
    ]
    res = run_bass_kernel_spmd(nc, in_maps, core_ids=list(range(N_CORES)))
    outs = [np.asarray(res.results[c]["out"]) for c in range(N_CORES)]
    full = np.concatenate(outs, axis=0)
    return full.reshape(x_shape[:-1] + (N_FEAT,)).astype(np.float32)



# revision 10
# speedup vs baseline: 1.8989x; 1.8989x over previous
"""Trainium2 Bass kernel for BlockdiagButterflyLinear.

Computes y = butterfly(x; w1, w2) + bias where
  tmp[b,k,j,y] = sum_i x[b, k*1024+i] * w1[k, j*48+y, i]
  out[b, 4l+j] = sum_{k,y} tmp[b,k,j,y] * w2[j, l, k*48+y] + bias[4l+j]

Sharding: data-parallel over the 8192 token rows across 8 NeuronCores
(1024 tokens/core); the small butterfly factors are replicated.

All device traffic is bf16 (the 2e-2 rel-err budget leaves ~50x margin):
x is cast + transposed on the host so no on-chip transposes are needed,
weights are pre-packed into their exact SBUF layouts, and the output is
written bf16 in j-major feature order and un-permuted/upcast on the host.
Per-core HBM traffic: 8.39 MB x + 8.39 MB out + 3.4 MB weights ~= 56 us
at the 360 B/ns DMA roofline; PE work is 131072 matmul rows ~= 55 us at
bf16 full rate, so the kernel is jointly DMA/PE-limited.

Per-core structure (two 512-token halves, th = 0/1):
  stage 1 (per k, j-pair): 8 accumulating matmuls contract i over 8x128
    with stationary w1 [128i, 96jy] and moving x^T [128i, 512tok] into
    PSUM [96, 512]; copies split the j-pair into t2[j][c][th] tiles
    [113, 512] (rows 0:48 = k even, 64:112 = k odd, 112 = ones row for
    the bias, 48:64 zero gap).
  stage 2 (per 128-token block m, j, l-half): 2 accumulating matmuls
    with stationary t2 [113, 128tok] and moving w2 [113, 512l] into
    PSUM [128, 512]; copies downcast into [128, 4096] bf16 out tiles
    (j-major feature order), DMA'd out in [128, 2048] halves.
"""

import sys

sys.path.insert(0, "/opt/trn_rl_repo")

from contextlib import ExitStack

import numpy as np
import ml_dtypes

import concourse.bacc as bacc
import concourse.bass as bass
import concourse.mybir as mybir
import concourse.tile as tile
from concourse.bass_utils import run_bass_kernel_spmd

F32 = mybir.dt.float32
BF16 = mybir.dt.bfloat16
NP_BF16 = ml_dtypes.bfloat16

N_CORES = 8
TOK = 1024  # tokens per core
N_FEAT = 4096
K, J, B1 = 4, 4, 48
TH = 512  # token half
R2 = 113  # stage-2 contraction rows: 48 + 16 gap + 48 + ones row

_PROGRAM = None


def _build_program() -> bass.Bass:
    nc = bacc.Bacc(None, target_bir_lowering=False)
    xs = nc.declare_dram_parameter("xs", [N_FEAT, TOK], BF16, isOutput=False)
    w1p = nc.declare_dram_parameter("w1p", [K, 128, 8, 224], BF16, isOutput=False)
    w2p = nc.declare_dram_parameter("w2p", [J, R2, 2, 1024], BF16, isOutput=False)
    out = nc.declare_dram_parameter("out", [TOK, N_FEAT], BF16, isOutput=True)

    with ExitStack() as ctx:
        tc = ctx.enter_context(tile.TileContext(nc))
        consts = ctx.enter_context(tc.tile_pool(name="consts", bufs=1))
        wpool = ctx.enter_context(tc.tile_pool(name="wpool", bufs=1))
        xpool = ctx.enter_context(tc.tile_pool(name="xpool", bufs=1))
        opool = ctx.enter_context(tc.tile_pool(name="opool", bufs=4))
        p1pool = ctx.enter_context(tc.tile_pool(name="p1pool", bufs=3, space="PSUM"))
        p2pool = ctx.enter_context(tc.tile_pool(name="p2pool", bufs=4, space="PSUM"))

        w1s = [wpool.tile([128, 8, 224], BF16, name=f"w1s_{k}") for k in range(K)]
        w2s = [wpool.tile([R2, 2, 1024], BF16, name=f"w2s_{j}") for j in range(J)]
        # t2[j][c][th]: stage-2 lhsT tiles. Rows 32:64 zeroed / 96:113 ones
        # once at startup; stage-1 copies then overwrite 0:48 and 64:112,
        # leaving the 48:64 zero gap and the 112 ones (bias) row.
        t2 = [
            [
                [consts.tile([R2, TH], BF16, name=f"t2_{j}_{c}_{th}") for th in (0, 1)]
                for c in (0, 1)
            ]
            for j in range(J)
        ]
        xk = [
            [xpool.tile([128, 8, TH], BF16, name=f"x_{k}_{th}") for th in (0, 1)]
            for k in range(K)
        ]

        for th in (0, 1):
            for c in (0, 1):
                for j in range(J):
                    nc.gpsimd.memset(t2[j][c][th][32:64, :], 0.0)
                    nc.gpsimd.memset(t2[j][c][th][96:R2, :], 1.0)

        def load_x(k, th):
            nc.sync.dma_start(
                xk[k][th][:],
                xs[
                    k * 1024 : (k + 1) * 1024, th * TH : (th + 1) * TH
                ].rearrange("(ic p) t -> p ic t", p=128),
            )

        for k in range(K):
            nc.sync.dma_start(w1s[k][:], w1p[k])
            load_x(k, 0)
        for j in range(J):
            nc.sync.dma_start(w2s[j][:], w2p[j])
        for k in range(K):
            load_x(k, 1)

        def s1(k, th):
            # p1 rows: 0:48 = j_even of the pair, 16-row gap (zero lhsT
            # columns, never read), 64:112 = j_odd — both copy sources
            # start at the 0/64 partition bases the BIR verifier requires.
            c = k // 2
            r0 = (k % 2) * 64
            for jp in range(2):
                p1 = p1pool.tile([112, TH], F32, tag="p1")
                for ic in range(8):
                    nc.tensor.matmul(
                        p1[:],
                        w1s[k][:, ic, jp * 112 : (jp + 1) * 112],
                        xk[k][th][:, ic, :],
                        start=(ic == 0),
                        stop=(ic == 7),
                    )
                ja, jb = 2 * jp, 2 * jp + 1
                if k % 2 == 0:
                    # partition-preserving copies go on DVE, shifts on ACT
                    nc.vector.tensor_copy(t2[ja][c][th][0:48, :], p1[0:48, :])
                    nc.scalar.copy(t2[jb][c][th][0:48, :], p1[64:112, :])
                else:
                    nc.scalar.copy(t2[ja][c][th][64:112, :], p1[0:48, :])
                    nc.vector.tensor_copy(t2[jb][c][th][64:112, :], p1[64:112, :])

        _rr = [0]
        _eng = [nc.vector.tensor_copy, nc.scalar.copy]

        def s2(m, th):
            outm = opool.tile([128, N_FEAT], BF16, tag="outm")
            for j in range(J):
                for lc in range(2):
                    p2 = p2pool.tile([128, TH], F32, tag="p2")
                    for c in range(2):
                        nc.tensor.matmul(
                            p2[:],
                            t2[j][c][th][:, m * 128 : (m + 1) * 128],
                            w2s[j][:, c, lc * 512 : (lc + 1) * 512],
                            start=(c == 0),
                            stop=(c == 1),
                        )
                    dst = outm[:, j * 1024 + lc * 512 : j * 1024 + (lc + 1) * 512]
                    _eng[_rr[0] % 2](dst, p2[:])
                    _rr[0] += 1
            row0 = th * TH + m * 128
            for h in range(2):
                nc.sync.dma_start(
                    out[row0 : row0 + 128, h * 2048 : (h + 1) * 2048],
                    outm[:, h * 2048 : (h + 1) * 2048],
                )

        for k in range(K):
            s1(k, 0)
        s1(0, 1)
        for m in range(4):
            s2(m, 0)
        for k in range(1, K):
            s1(k, 1)
        for m in range(4):
            s2(m, 1)

    nc.compile()
    nc.finalize()
    return nc


def _get_program() -> bass.Bass:
    global _PROGRAM
    if _PROGRAM is None:
        _PROGRAM = _build_program()
    return _PROGRAM


def _prep_weights(w1, w2, b):
    # w1p[k, p, ic, jp*112 + q] = w1[k, (2jp + (q >= 64))*48 + q%64, ic*128+p]
    # with 16 zero columns at 48:64 of each 112-wide j-pair group.
    w1t = w1.transpose(0, 2, 1)  # (k, i, f)
    w1pad = np.zeros((K, 1024, 224), np.float32)
    for jp in range(2):
        w1pad[:, :, jp * 112 : jp * 112 + 48] = w1t[:, :, jp * 96 : jp * 96 + 48]
        w1pad[:, :, jp * 112 + 64 : jp * 112 + 112] = w1t[
            :, :, jp * 96 + 48 : jp * 96 + 96
        ]
    w1p = np.ascontiguousarray(
        w1pad.reshape(K, 8, 128, 224).transpose(0, 2, 1, 3)
    ).astype(NP_BF16)
    # w2p[j, r, c, l]: rows 0:48 = w2[j, l, 96c+y].T, 64:112 = the k-odd
    # half, 112 = bias (only on the c=1 chunk), gap rows zero.
    w2p = np.zeros((J, R2, 2, 1024), np.float32)
    for j in range(J):
        w2j = w2[j]  # (1024 l, 192 ky)
        for c in range(2):
            w2p[j, 0:48, c, :] = w2j[:, 96 * c : 96 * c + 48].T
            w2p[j, 64:112, c, :] = w2j[:, 96 * c + 48 : 96 * c + 96].T
        w2p[j, 112, 1, :] = b[j::J]  # bias[4l+j]
    return w1p, w2p.astype(NP_BF16)


def kernel(x, w1_bfly, w2_bfly, bias):
    x = np.asarray(x, dtype=np.float32)
    w1 = np.asarray(w1_bfly, dtype=np.float32)
    w2 = np.asarray(w2_bfly, dtype=np.float32)
    b = np.asarray(bias, dtype=np.float32)

    x_shape = x.shape
    xb = np.ascontiguousarray(x).reshape(-1, N_FEAT).astype(NP_BF16)
    w1p, w2p = _prep_weights(w1, w2, b)

    nc = _get_program()
    in_maps = [
        {
            "xs": np.ascontiguousarray(xb[c * TOK : (c + 1) * TOK].T),
            "w1p": w1p,
            "w2p": w2p,
        }
        for c in range(N_CORES)
    ]
    res = run_bass_kernel_spmd(nc, in_maps, core_ids=list(range(N_CORES)))
    outs = [np.asarray(res.results[c]["out"]) for c in range(N_CORES)]
    full = np.concatenate(outs, axis=0)  # (8192, 4096) bf16, j-major feats
    full = (
        full.reshape(-1, J, 1024).transpose(0, 2, 1).reshape(-1, N_FEAT)
    ).astype(np.float32)
    return full.reshape(x_shape[:-1] + (N_FEAT,))


# revision 38
# speedup vs baseline: 2.0201x; 1.0638x over previous
"""Trainium2 Bass kernel for BlockdiagButterflyLinear.

Computes y = butterfly(x; w1, w2) + bias where
  tmp[b,k,j,y] = sum_i x[b, k*1024+i] * w1[k, j*48+y, i]
  out[b, 4l+j] = sum_{k,y} tmp[b,k,j,y] * w2[j, l, k*48+y] + bias[4l+j]

Sharding: data-parallel over the 8192 token rows across 8 NeuronCores
(1024 tokens/core); the small butterfly factors are replicated.

All device traffic is bf16 (the 2e-2 rel-err budget leaves ~50x margin):
x is cast + transposed on the host so no on-chip transposes are needed,
weights are pre-packed into their exact SBUF layouts, and the output is
written bf16 in j-major feature order and un-permuted/upcast on the host.
Per-core HBM traffic: 8.39 MB x + 8.39 MB out + 3.4 MB weights ~= 56 us
at the 360 B/ns DMA roofline; PE work is 131072 matmul rows ~= 55 us at
bf16 full rate, so the kernel is jointly DMA/PE-limited.

Per-core structure (two 512-token halves, th = 0/1):
  stage 1 (per k, j-pair): 8 accumulating matmuls contract i over 8x128
    with stationary w1 [128i, 96jy] and moving x^T [128i, 512tok] into
    PSUM [96, 512]; copies split the j-pair into t2[j][c][th] tiles
    [113, 512] (rows 0:48 = k even, 64:112 = k odd, 112 = ones row for
    the bias, 48:64 zero gap).
  stage 2 (per 128-token block m, j, l-half): 2 accumulating matmuls
    with stationary t2 [113, 128tok] and moving w2 [113, 512l] into
    PSUM [128, 512]; copies downcast into [128, 4096] bf16 out tiles
    (j-major feature order), DMA'd out in [128, 2048] halves.
"""

import sys

sys.path.insert(0, "/opt/trn_rl_repo")

from contextlib import ExitStack

import numpy as np
import ml_dtypes

import concourse.bacc as bacc
import concourse.bass as bass
import concourse.mybir as mybir
import concourse.tile as tile
from concourse.bass_utils import run_bass_kernel_spmd

F32 = mybir.dt.float32
BF16 = mybir.dt.bfloat16
NP_BF16 = ml_dtypes.bfloat16

N_CORES = 8
TOK = 1024  # tokens per core
N_FEAT = 4096
K, J, B1 = 4, 4, 48
TH = 512  # token half
R2 = 113  # stage-2 contraction rows: 48 + 16 gap + 48 + ones row
N_WARM = 27  # PE warm-up matmuls (256 rows each), tuned against TimelineSim

_PROGRAM = None


def _build_program() -> bass.Bass:
    nc = bacc.Bacc(None, target_bir_lowering=False)
    xs = nc.declare_dram_parameter("xs", [N_FEAT, TOK], BF16, isOutput=False)
    w1p = nc.declare_dram_parameter("w1p", [K, 128, 8, 224], BF16, isOutput=False)
    w2p = nc.declare_dram_parameter("w2p", [J, R2, 2, 1024], BF16, isOutput=False)
    out = nc.declare_dram_parameter("out", [TOK, N_FEAT], BF16, isOutput=True)

    with ExitStack() as ctx:
        tc = ctx.enter_context(tile.TileContext(nc))
        consts = ctx.enter_context(tc.tile_pool(name="consts", bufs=1))
        wpool = ctx.enter_context(tc.tile_pool(name="wpool", bufs=1))
        xpool = ctx.enter_context(tc.tile_pool(name="xpool", bufs=1))
        opool = ctx.enter_context(tc.tile_pool(name="opool", bufs=4))
        p1pool = ctx.enter_context(tc.tile_pool(name="p1pool", bufs=4, space="PSUM"))
        p2pool = ctx.enter_context(tc.tile_pool(name="p2pool", bufs=4, space="PSUM"))

        # w1 split per ic-half so each stage-1 half-unit only waits on the
        # half of the weight/x stream it actually reads
        w1s = [
            [wpool.tile([128, 4, 224], BF16, name=f"w1s_{k}_{h}") for h in (0, 1)]
            for k in range(K)
        ]
        w2s = [wpool.tile([R2, 2, 1024], BF16, name=f"w2s_{j}") for j in range(J)]
        # t2[j][c][th]: stage-2 lhsT tiles. Rows 32:64 zeroed / 96:113 ones
        # once at startup; stage-1 copies then overwrite 0:48 and 64:112,
        # leaving the 48:64 zero gap and the 112 ones (bias) row.
        t2 = [
            [
                [consts.tile([R2, TH], BF16, name=f"t2_{j}_{c}_{th}") for th in (0, 1)]
                for c in (0, 1)
            ]
            for j in range(J)
        ]
        xk = [
            [
                [
                    xpool.tile([128, 4, TH], BF16, name=f"x_{k}_{th}_{h}")
                    for h in (0, 1)
                ]
                for th in (0, 1)
            ]
            for k in range(K)
        ]

        # PE warm-up: the first ~27 tensor-engine matmuls run at reduced
        # pstate. Real matmuls can't start until ~6.7 us (input DMA stream),
        # so burn the slow instructions on throwaway matmuls first.
        warm = consts.tile([128, 256], BF16, name="warm")
        # warm PSUM target shares p1pool's rotation: its buffer is reused by
        # the second stage-1 unit, which starts only after warm-up ends
        pwarm = p1pool.tile([112, TH], F32, tag="p1", name="pwarm")
        nc.vector.memset(warm[:], 0.0)
        for _ in range(N_WARM):
            nc.tensor.matmul(
                pwarm[:, 0:256], warm[:, 0:112], warm[:], start=True, stop=True
            )

        for th in (0, 1):
            for c in (0, 1):
                for j in range(J):
                    nc.gpsimd.memset(t2[j][c][th][32:64, :], 0.0)
                    nc.gpsimd.memset(t2[j][c][th][96:R2, :], 1.0)

        def load_x(k, th, h):
            r0 = k * 1024 + h * 512
            nc.sync.dma_start(
                xk[k][th][h][:],
                xs[r0 : r0 + 512, th * TH : (th + 1) * TH].rearrange(
                    "(ic p) t -> p ic t", p=128
                ),
            )

        def load_w1(k, h):
            nc.sync.dma_start(w1s[k][h][:], w1p[k, :, h * 4 : (h + 1) * 4])

        def load_w2(j, c):
            nc.sync.dma_start(w2s[j][:, c, :], w2p[j, :, c, :])

        # Issue order tuned so PE (full rate after warm-up, one 1.71 us
        # half-unit per x half-tile) never waits on a tile that hasn't
        # landed: th1 x loads interleave into the th0 stream; w2 halves
        # slot in just ahead of when stage 2 reads them.
        for k, th in [(0, 0), (1, 0), (0, 1), (2, 0), (1, 1), (3, 0), (2, 1), (3, 1)]:
            for h in (0, 1):
                if th == 0:
                    load_w1(k, h)
                load_x(k, th, h)
        for j in range(J):
            load_w2(j, 0)
            load_w2(j, 1)

        p1live = {}

        def s1a(k, th):
            # first ic-half of both j-pair accumulations; p1 rows: 0:48 =
            # j_even, 16-row gap (zero lhsT columns), 64:112 = j_odd — copy
            # sources land on the 0/64 partition bases the verifier requires.
            ps = [
                p1pool.tile([112, TH], F32, tag="p1", name=f"p1_{k}_{th}_{jp}")
                for jp in range(2)
            ]
            p1live[(k, th)] = ps
            for jp in range(2):
                for ic in range(4):
                    nc.tensor.matmul(
                        ps[jp][:],
                        w1s[k][0][:, ic, jp * 112 : (jp + 1) * 112],
                        xk[k][th][0][:, ic, :],
                        start=(ic == 0),
                        stop=False,
                    )

        def s1b(k, th):
            c = k // 2
            ps = p1live.pop((k, th))
            for jp in range(2):
                for ic in range(4):
                    nc.tensor.matmul(
                        ps[jp][:],
                        w1s[k][1][:, ic, jp * 112 : (jp + 1) * 112],
                        xk[k][th][1][:, ic, :],
                        start=False,
                        stop=(ic == 3),
                    )
                ja, jb = 2 * jp, 2 * jp + 1
                if k % 2 == 0:
                    # partition-preserving copies go on DVE, shifts on ACT
                    nc.vector.tensor_copy(t2[ja][c][th][0:48, :], ps[jp][0:48, :])
                    nc.scalar.copy(t2[jb][c][th][0:48, :], ps[jp][64:112, :])
                else:
                    nc.scalar.copy(t2[ja][c][th][64:112, :], ps[jp][0:48, :])
                    nc.vector.tensor_copy(
                        t2[jb][c][th][64:112, :], ps[jp][64:112, :]
                    )

        _rr = [0]
        _eng = [nc.vector.tensor_copy, nc.scalar.copy]

        def s2(m, th, last=False):
            outm = opool.tile([128, N_FEAT], BF16, tag="outm")
            row0 = th * TH + m * 128
            for j in range(J):
                for lc in range(2):
                    p2 = p2pool.tile([128, TH], F32, tag="p2")
                    for c in range(2):
                        nc.tensor.matmul(
                            p2[:],
                            t2[j][c][th][:, m * 128 : (m + 1) * 128],
                            w2s[j][:, c, lc * 512 : (lc + 1) * 512],
                            start=(c == 0),
                            stop=(c == 1),
                        )
                    dst = outm[:, j * 1024 + lc * 512 : j * 1024 + (lc + 1) * 512]
                    _eng[_rr[0] % 2](dst, p2[:])
                    _rr[0] += 1
                if last:
                    # final unit: per-j quarter stores (eighths for the very
                    # last j) shorten the trailing copy->DMA latency chain
                    if j < J - 1:
                        nc.sync.dma_start(
                            out[row0 : row0 + 128, j * 1024 : (j + 1) * 1024],
                            outm[:, j * 1024 : (j + 1) * 1024],
                        )
                    else:
                        for lc in range(2):
                            c0 = j * 1024 + lc * 512
                            nc.sync.dma_start(
                                out[row0 : row0 + 128, c0 : c0 + 512],
                                outm[:, c0 : c0 + 512],
                            )
            if not last:
                for h in range(2):
                    nc.sync.dma_start(
                        out[row0 : row0 + 128, h * 2048 : (h + 1) * 2048],
                        outm[:, h * 2048 : (h + 1) * 2048],
                    )

        # PE unit order matched to the DMA arrival order above: th1 stage-1
        # half-units fill the gaps while th0's x stream finishes; stage 2
        # runs last with its out DMAs riding behind the copies.
        for k, th in [(0, 0), (1, 0), (0, 1), (2, 0), (1, 1), (3, 0), (2, 1), (3, 1)]:
            s1a(k, th)
            s1b(k, th)
        for th in (0, 1):
            for m in range(4):
                s2(m, th, last=(th == 1 and m == 3))

    nc.compile()
    nc.finalize()
    return nc


def _get_program() -> bass.Bass:
    global _PROGRAM
    if _PROGRAM is None:
        _PROGRAM = _build_program()
    return _PROGRAM


def _prep_weights(w1, w2, b):
    # w1p[k, p, ic, jp*112 + q] = w1[k, (2jp + (q >= 64))*48 + q%64, ic*128+p]
    # with 16 zero columns at 48:64 of each 112-wide j-pair group.
    w1t = w1.transpose(0, 2, 1)  # (k, i, f)
    w1pad = np.zeros((K, 1024, 224), np.float32)
    for jp in range(2):
        w1pad[:, :, jp * 112 : jp * 112 + 48] = w1t[:, :, jp * 96 : jp * 96 + 48]
        w1pad[:, :, jp * 112 + 64 : jp * 112 + 112] = w1t[
            :, :, jp * 96 + 48 : jp * 96 + 96
        ]
    w1p = np.ascontiguousarray(
        w1pad.reshape(K, 8, 128, 224).transpose(0, 2, 1, 3)
    ).astype(NP_BF16)
    # w2p[j, r, c, l]: rows 0:48 = w2[j, l, 96c+y].T, 64:112 = the k-odd
    # half, 112 = bias (only on the c=1 chunk), gap rows zero.
    w2p = np.zeros((J, R2, 2, 1024), np.float32)
    for j in range(J):
        w2j = w2[j]  # (1024 l, 192 ky)
        for c in range(2):
            w2p[j, 0:48, c, :] = w2j[:, 96 * c : 96 * c + 48].T
            w2p[j, 64:112, c, :] = w2j[:, 96 * c + 48 : 96 * c + 96].T
        w2p[j, 112, 1, :] = b[j::J]  # bias[4l+j]
    return w1p, w2p.astype(NP_BF16)


def kernel(x, w1_bfly, w2_bfly, bias):
    x = np.asarray(x, dtype=np.float32)
    w1 = np.asarray(w1_bfly, dtype=np.float32)
    w2 = np.asarray(w2_bfly, dtype=np.float32)
    b = np.asarray(bias, dtype=np.float32)

    x_shape = x.shape
    xb = np.ascontiguousarray(x).reshape(-1, N_FEAT).astype(NP_BF16)
    w1p, w2p = _prep_weights(w1, w2, b)

    nc = _get_program()
    in_maps = [
        {
            "xs": np.ascontiguousarray(xb[c * TOK : (c + 1) * TOK].T),
            "w1p": w1p,
            "w2p": w2p,
        }
        for c in range(N_CORES)
    ]
    res = run_bass_kernel_spmd(nc, in_maps, core_ids=list(range(N_CORES)))
    outs = [np.asarray(res.results[c]["out"]) for c in range(N_CORES)]
    full = np.concatenate(outs, axis=0)  # (8192, 4096) bf16, j-major feats
    full = (
        full.reshape(-1, J, 1024).transpose(0, 2, 1).reshape(-1, N_FEAT)
    ).astype(np.float32)
    return full.reshape(x_shape[:-1] + (N_FEAT,))


# revision 51
# speedup vs baseline: 2.0320x; 1.0059x over previous
"""Trainium2 Bass kernel for BlockdiagButterflyLinear.

Computes y = butterfly(x; w1, w2) + bias where
  tmp[b,k,j,y] = sum_i x[b, k*1024+i] * w1[k, j*48+y, i]
  out[b, 4l+j] = sum_{k,y} tmp[b,k,j,y] * w2[j, l, k*48+y] + bias[4l+j]

Sharding: data-parallel over the 8192 token rows across 8 NeuronCores
(1024 tokens/core); the small butterfly factors are replicated.

All device traffic is bf16 (the 2e-2 rel-err budget leaves ~50x margin):
x is cast + transposed on the host so no on-chip transposes are needed,
weights are pre-packed into their exact SBUF layouts, and the output is
written bf16 in j-major feature order and un-permuted/upcast on the host.
Per-core HBM traffic: 8.39 MB x + 8.39 MB out + 3.4 MB weights ~= 56 us
at the 360 B/ns DMA roofline; PE work is 131072 matmul rows ~= 55 us at
bf16 full rate, so the kernel is jointly DMA/PE-limited.

Per-core structure (two 512-token halves, th = 0/1):
  stage 1 (per k, j-pair): 8 accumulating matmuls contract i over 8x128
    with stationary w1 [128i, 96jy] and moving x^T [128i, 512tok] into
    PSUM [96, 512]; copies split the j-pair into t2[j][c][th] tiles
    [113, 512] (rows 0:48 = k even, 64:112 = k odd, 112 = ones row for
    the bias, 48:64 zero gap).
  stage 2 (per 128-token block m, j, l-half): 2 accumulating matmuls
    with stationary t2 [113, 128tok] and moving w2 [113, 512l] into
    PSUM [128, 512]; copies downcast into [128, 4096] bf16 out tiles
    (j-major feature order), DMA'd out in [128, 2048] halves.
"""

import sys

sys.path.insert(0, "/opt/trn_rl_repo")

from contextlib import ExitStack

import numpy as np
import ml_dtypes

import concourse.bacc as bacc
import concourse.bass as bass
import concourse.mybir as mybir
import concourse.tile as tile
from concourse.bass_utils import run_bass_kernel_spmd

F32 = mybir.dt.float32
BF16 = mybir.dt.bfloat16
NP_BF16 = ml_dtypes.bfloat16

N_CORES = 8
TOK = 1024  # tokens per core
N_FEAT = 4096
K, J, B1 = 4, 4, 48
TH = 512  # token half
R2 = 113  # stage-2 contraction rows: 48 + 16 gap + 48 + ones row
N_WARM = 27  # PE warm-up matmuls (256 rows each), tuned against TimelineSim

_PROGRAM = None


def _build_program() -> bass.Bass:
    nc = bacc.Bacc(None, target_bir_lowering=False)
    xs = nc.declare_dram_parameter("xs", [N_FEAT, TOK], BF16, isOutput=False)
    w1p = nc.declare_dram_parameter("w1p", [K, 128, 8, 224], BF16, isOutput=False)
    w2p = nc.declare_dram_parameter("w2p", [J, R2, 2, 1024], BF16, isOutput=False)
    out = nc.declare_dram_parameter("out", [TOK, N_FEAT], BF16, isOutput=True)

    with ExitStack() as ctx:
        tc = ctx.enter_context(tile.TileContext(nc))
        consts = ctx.enter_context(tc.tile_pool(name="consts", bufs=1))
        wpool = ctx.enter_context(tc.tile_pool(name="wpool", bufs=1))
        xpool = ctx.enter_context(tc.tile_pool(name="xpool", bufs=1))
        opool = ctx.enter_context(tc.tile_pool(name="opool", bufs=4))
        p1pool = ctx.enter_context(tc.tile_pool(name="p1pool", bufs=4, space="PSUM"))
        p2pool = ctx.enter_context(tc.tile_pool(name="p2pool", bufs=4, space="PSUM"))

        # w1 split per ic-half so each stage-1 half-unit only waits on the
        # half of the weight/x stream it actually reads
        w1s = [
            [wpool.tile([128, 4, 224], BF16, name=f"w1s_{k}_{h}") for h in (0, 1)]
            for k in range(K)
        ]
        w2s = [wpool.tile([R2, 2, 1024], BF16, name=f"w2s_{j}") for j in range(J)]
        # t2[j][c][th]: stage-2 lhsT tiles. Rows 32:64 zeroed / 96:113 ones
        # once at startup; stage-1 copies then overwrite 0:48 and 64:112,
        # leaving the 48:64 zero gap and the 112 ones (bias) row.
        t2 = [
            [
                [consts.tile([R2, TH], BF16, name=f"t2_{j}_{c}_{th}") for th in (0, 1)]
                for c in (0, 1)
            ]
            for j in range(J)
        ]
        xk = [
            [
                [
                    xpool.tile([128, 4, TH], BF16, name=f"x_{k}_{th}_{h}")
                    for h in (0, 1)
                ]
                for th in (0, 1)
            ]
            for k in range(K)
        ]

        # PE warm-up: the first ~27 tensor-engine matmuls run at reduced
        # pstate. Real matmuls can't start until ~6.7 us (input DMA stream),
        # so burn the slow instructions on throwaway matmuls first.
        warm = consts.tile([128, 256], BF16, name="warm")
        # warm PSUM target shares p1pool's rotation: its buffer is reused by
        # the second stage-1 unit, which starts only after warm-up ends
        pwarm = p1pool.tile([112, TH], F32, tag="p1", name="pwarm")
        nc.vector.memset(warm[:], 0.0)
        for _ in range(N_WARM):
            nc.tensor.matmul(
                pwarm[:, 0:64], warm[:, 0:112], warm[:, 0:64], start=True, stop=True
            )

        for th in (0, 1):
            for c in (0, 1):
                for j in range(J):
                    nc.gpsimd.memset(t2[j][c][th][32:64, :], 0.0)
                    nc.gpsimd.memset(t2[j][c][th][96:R2, :], 1.0)

        def load_x(k, th, h):
            r0 = k * 1024 + h * 512
            nc.sync.dma_start(
                xk[k][th][h][:],
                xs[r0 : r0 + 512, th * TH : (th + 1) * TH].rearrange(
                    "(ic p) t -> p ic t", p=128
                ),
            )

        def load_w1(k, h, eng=None):
            (eng or nc.sync).dma_start(w1s[k][h][:], w1p[k, :, h * 4 : (h + 1) * 4])

        def load_w2(j, c):
            nc.sync.dma_start(w2s[j][:, c, :], w2p[j, :, c, :])

        # Issue order tuned so PE (full rate after warm-up, one 1.71 us
        # half-unit per x half-tile) never waits on a tile that hasn't
        # landed: th1 x loads interleave into the th0 stream; w2 halves
        # slot in just ahead of when stage 2 reads them.
        load_w1(0, 0, eng=nc.scalar)
        load_x(0, 0, 0)
        load_w1(0, 1)
        load_x(0, 0, 1)
        for k, th in [(1, 0), (0, 1), (2, 0), (1, 1), (3, 0), (2, 1)]:
            for h in (0, 1):
                if th == 0:
                    load_w1(k, h)
                load_x(k, th, h)
        load_x(3, 1, 0)
        load_x(3, 1, 1)
        for j in range(J):
            load_w2(j, 0)
            load_w2(j, 1)

        p1live = {}

        def s1a(k, th):
            # first ic-half of both j-pair accumulations; p1 rows: 0:48 =
            # j_even, 16-row gap (zero lhsT columns), 64:112 = j_odd — copy
            # sources land on the 0/64 partition bases the verifier requires.
            ps = [
                p1pool.tile([112, TH], F32, tag="p1", name=f"p1_{k}_{th}_{jp}")
                for jp in range(2)
            ]
            p1live[(k, th)] = ps
            for jp in range(2):
                for ic in range(4):
                    nc.tensor.matmul(
                        ps[jp][:],
                        w1s[k][0][:, ic, jp * 112 : (jp + 1) * 112],
                        xk[k][th][0][:, ic, :],
                        start=(ic == 0),
                        stop=False,
                    )

        def s1b(k, th):
            c = k // 2
            ps = p1live.pop((k, th))
            for jp in range(2):
                for ic in range(4):
                    nc.tensor.matmul(
                        ps[jp][:],
                        w1s[k][1][:, ic, jp * 112 : (jp + 1) * 112],
                        xk[k][th][1][:, ic, :],
                        start=False,
                        stop=(ic == 3),
                    )
                ja, jb = 2 * jp, 2 * jp + 1
                if k % 2 == 0:
                    # partition-preserving copies go on DVE, shifts on ACT
                    nc.vector.tensor_copy(t2[ja][c][th][0:48, :], ps[jp][0:48, :])
                    nc.scalar.copy(t2[jb][c][th][0:48, :], ps[jp][64:112, :])
                else:
                    nc.scalar.copy(t2[ja][c][th][64:112, :], ps[jp][0:48, :])
                    nc.vector.tensor_copy(
                        t2[jb][c][th][64:112, :], ps[jp][64:112, :]
                    )

        _rr = [0]
        _eng = [nc.vector.tensor_copy, nc.scalar.copy]

        outlive = {}

        def s2(m, th, js=range(J), last=False):
            if (m, th) in outlive:
                outm = outlive.pop((m, th))
            else:
                outm = opool.tile([128, N_FEAT], BF16, tag="outm", name=f"o_{m}_{th}")
                outlive[(m, th)] = outm
            row0 = th * TH + m * 128
            for j in js:
                for lc in range(2):
                    p2 = p2pool.tile([128, TH], F32, tag="p2")
                    for c in range(2):
                        nc.tensor.matmul(
                            p2[:],
                            t2[j][c][th][:, m * 128 : (m + 1) * 128],
                            w2s[j][:, c, lc * 512 : (lc + 1) * 512],
                            start=(c == 0),
                            stop=(c == 1),
                        )
                    dst = outm[:, j * 1024 + lc * 512 : j * 1024 + (lc + 1) * 512]
                    _eng[_rr[0] % 2](dst, p2[:])
                    _rr[0] += 1
                if last:
                    # final unit: per-j quarter stores (eighths for the very
                    # last j) shorten the trailing copy->DMA latency chain
                    if j < J - 1:
                        nc.sync.dma_start(
                            out[row0 : row0 + 128, j * 1024 : (j + 1) * 1024],
                            outm[:, j * 1024 : (j + 1) * 1024],
                        )
                    else:
                        for lc in range(2):
                            c0 = j * 1024 + lc * 512
                            nc.sync.dma_start(
                                out[row0 : row0 + 128, c0 : c0 + 512],
                                outm[:, c0 : c0 + 512],
                            )
            if not last:
                for jq in js:
                    nc.sync.dma_start(
                        out[row0 : row0 + 128, jq * 1024 : (jq + 1) * 1024],
                        outm[:, jq * 1024 : (jq + 1) * 1024],
                    )

        # PE unit order matched to the DMA arrival order above: th1 stage-1
        # half-units fill the gaps while th0's x stream finishes; stage 2
        # runs last with its out DMAs riding behind the copies.
        for k, th in [(0, 0), (1, 0), (0, 1), (2, 0), (1, 1), (3, 0), (2, 1)]:
            s1a(k, th)
            s1b(k, th)
        s1a(3, 1)
        s1b(3, 1)
        for m in range(4):
            s2(m, 0)
        for m in range(4):
            s2(m, 1, last=(m == 3))

    nc.compile()
    nc.finalize()
    return nc


def _get_program() -> bass.Bass:
    global _PROGRAM
    if _PROGRAM is None:
        _PROGRAM = _build_program()
    return _PROGRAM


def _prep_weights(w1, w2, b):
    # w1p[k, p, ic, jp*112 + q] = w1[k, (2jp + (q >= 64))*48 + q%64, ic*128+p]
    # with 16 zero columns at 48:64 of each 112-wide j-pair group.
    w1t = w1.transpose(0, 2, 1)  # (k, i, f)
    w1pad = np.zeros((K, 1024, 224), np.float32)
    for jp in range(2):
        w1pad[:, :, jp * 112 : jp * 112 + 48] = w1t[:, :, jp * 96 : jp * 96 + 48]
        w1pad[:, :, jp * 112 + 64 : jp * 112 + 112] = w1t[
            :, :, jp * 96 + 48 : jp * 96 + 96
        ]
    w1p = np.ascontiguousarray(
        w1pad.reshape(K, 8, 128, 224).transpose(0, 2, 1, 3)
    ).astype(NP_BF16)
    # w2p[j, r, c, l]: rows 0:48 = w2[j, l, 96c+y].T, 64:112 = the k-odd
    # half, 112 = bias (only on the c=1 chunk), gap rows zero.
    w2p = np.zeros((J, R2, 2, 1024), np.float32)
    for j in range(J):
        w2j = w2[j]  # (1024 l, 192 ky)
        for c in range(2):
            w2p[j, 0:48, c, :] = w2j[:, 96 * c : 96 * c + 48].T
            w2p[j, 64:112, c, :] = w2j[:, 96 * c + 48 : 96 * c + 96].T
        w2p[j, 112, 1, :] = b[j::J]  # bias[4l+j]
    return w1p, w2p.astype(NP_BF16)


def kernel(x, w1_bfly, w2_bfly, bias):
    x = np.asarray(x, dtype=np.float32)
    w1 = np.asarray(w1_bfly, dtype=np.float32)
    w2 = np.asarray(w2_bfly, dtype=np.float32)
    b = np.asarray(bias, dtype=np.float32)

    x_shape = x.shape
    xb = np.ascontiguousarray(x).reshape(-1, N_FEAT).astype(NP_BF16)
    w1p, w2p = _prep_weights(w1, w2, b)

    nc = _get_program()
    in_maps = [
        {
            "xs": np.ascontiguousarray(xb[c * TOK : (c + 1) * TOK].T),
            "w1p": w1p,
            "w2p": w2p,
        }
        for c in range(N_CORES)
    ]
    res = run_bass_kernel_spmd(nc, in_maps, core_ids=list(range(N_CORES)))
    outs = [np.asarray(res.results[c]["out"]) for c in range(N_CORES)]
    full = np.concatenate(outs, axis=0)  # (8192, 4096) bf16, j-major feats
    full = (
        full.reshape(-1, J, 1024).transpose(0, 2, 1).reshape(-1, N_FEAT)
    ).astype(np.float32)
    return full.reshape(x_shape[:-1] + (N_FEAT,))
